# revision 24
# baseline (speedup 1.0000x reference)
"""MultiHeadAttention TRN2 kernel — hybrid sharding: 2 batch-groups x 4
head-groups over 8 cores. Core c = (bg, hg) with bg = c//4, hg = c%4 owns
batches {2bg, 2bg+1} and heads {4hg..4hg+3} == feature columns
hg*256:(hg+1)*256 of Wq/Wk/Wv and rows hg*256:(hg+1)*256 of Wo.

vs pure head-parallel this halves the dominant DMA traffic (each core loads
x for 2 of 4 batches instead of all) while keeping per-core PE/ACT/DVE work
identical, and keeps >=2 local batches so projections of batch b+1 pipeline
under attention of batch b.

Device math (per core), all matmuls bf16 with f32 PSUM accumulation:
  qT/kT = (Wq_c^T x^T + bq_c)          feature-major [2x128, S] per batch
  v     = x^T^T Wv_c                   position-major, 65-wide blocks per
                                        (kv-tile, head); col 64 = ones so AV
                                        accumulates the softmax denominator
  scoresT[kv, q] = kT^T qT             per (b, h), exp via ACT with scale=1/8
  AV (position-major, full 128 PSUM partitions):
    ap[q, j] = sum_kv e[kv, q] v_aug[kv, j]
  attn_pm[q, d] = ap[q, d] / ap[q, 64]  per-partition scalar mul on DVE
  transpose attn_pm -> feature-major via PE identity transpose (f32)
  out_partial[q, e] = sum_hp attn_fm^T Wo_chunk   bf16 out, host sums the 4
                                        head-group partials per batch-group
"""

import sys

sys.path.insert(0, "/opt/trn_rl_repo")

import numpy as np
import ml_dtypes

import concourse.bass as bass
from concourse import bacc
import concourse.mybir as mybir
from concourse.tile import TileContext
from concourse.bass_utils import run_bass_kernel_spmd

BF16 = mybir.dt.bfloat16
F32 = mybir.dt.float32
F8 = mybir.dt.float8e4
AF = mybir.ActivationFunctionType
DR = mybir.MatmulPerfMode.DoubleRow

EMBED = 1024
HEADS = 16
HEAD_DIM = 64
N_CORES = 8
BPC = 2  # batches per core (batch-group size)
HPC = 4  # heads per core
DC = HPC * HEAD_DIM  # 256 feature columns per core
FCH = DC // 128  # feature chunks of 128
NEC = 8  # contraction chunks of 128 over EMBED


def build_nc(B=4, S=2048, lowering=False, iters=1, ablate=None):
    ROWS = BPC * S  # rows owned by this core's batch-group
    NQC = S // 512  # q chunks per (b, h)
    NKV = S // 128  # kv tiles per batch
    NKV2 = NKV // 2
    nc = bacc.Bacc("TRN2", target_bir_lowering=lowering)

    # q/k path in fp8e4 (chunk-major [128, NEC, ROWS] flattened) for
    # DoubleRow matmuls; v path stays bf16 feature-major.
    qT_d = nc.declare_dram_parameter("qT8", [128, NEC * ROWS], F8, isOutput=False)
    kT_d = nc.declare_dram_parameter("kT8", [128, NEC * ROWS], F8, isOutput=False)
    vT_d = nc.declare_dram_parameter("vT", [EMBED, ROWS], BF16, isOutput=False)
    wq_d = nc.declare_dram_parameter("wq8", [128, NEC * DC], F8, isOutput=False)
    wk_d = nc.declare_dram_parameter("wk8", [128, NEC * DC], F8, isOutput=False)
    wv_d = nc.declare_dram_parameter("wv", [128, NEC * DC], BF16, isOutput=False)
    bq_d = nc.declare_dram_parameter("bq", [128, FCH], F32, isOutput=False)
    bk_d = nc.declare_dram_parameter("bk", [128, FCH], F32, isOutput=False)
    wo_d = nc.declare_dram_parameter("wo", [128, FCH * EMBED], BF16, isOutput=False)
    id_d = nc.declare_dram_parameter("ident", [128, 128], BF16, isOutput=False)
    out_d = nc.declare_dram_parameter("out", [ROWS, EMBED], BF16, isOutput=True)

    with TileContext(nc) as tc:
        with (
            tc.tile_pool(name="const", bufs=1) as cpool,
            tc.tile_pool(name="big", bufs=2) as big,
            tc.tile_pool(name="xin", bufs=32) as xin,
            tc.tile_pool(name="expp", bufs=14) as expp,
            tc.tile_pool(name="apm", bufs=16) as apmp,
            tc.tile_pool(name="ev", bufs=8) as evp,
            tc.tile_pool(name="ot", bufs=4) as otp,
            tc.tile_pool(name="ps", bufs=1, space="PSUM") as ps,
        ):
            # --- weights / constants ---
            wq_sb = cpool.tile([128, NEC * DC], F8, tag="wq")
            wk_sb = cpool.tile([128, NEC * DC], F8, tag="wk")
            wv_sb = cpool.tile([128, NEC * DC], BF16, tag="wv")
            wo_sb = cpool.tile([128, FCH * EMBED], BF16, tag="wo")
            bq_sb = cpool.tile([128, FCH], F32, tag="bq")
            bk_sb = cpool.tile([128, FCH], F32, tag="bk")
            id_sb = cpool.tile([128, 128], BF16, tag="ident")

            # --- PSUM bank plan (8 banks total) ---
            # spsA (4 banks) / spsB (2 banks): alternating score tiles so one
            # exp instruction covers 2048/1024 columns (ACT per-instruction
            # overhead is ~270ns; fewer+bigger exps is the ACT win).
            # pb (1 bank): qk-proj psum pt [0:512].
            # ob (1 bank): po0 [0:256] / po1 [256:512] outproj ping-pong;
            #   ap0 [0:65], ap1 [65:130] AV accumulators (overlap po0 —
            #   order-safe: AV phase and outproj phase never overlap for the
            #   same region window); pv [256:384] v-proj psum (overlaps po1,
            #   order-safe for the same reason); tp0/tp1 [384:448]/[448:512]
            #   bitcast-bf16 transpose outputs (overlap po1, drained by afm
            #   copies before po1's writes in program order).
            pb_t = ps.tile([128, 512], F32, tag="pb", bufs=1, name="pb")
            ob_t = ps.tile([128, 512], F32, tag="ob", bufs=1, name="ob")

            qT_sb, kT_sb, v_sb = {}, {}, {}

            def alloc_batch(b):
                qT_sb[b] = big.tile([128, FCH * S], BF16, tag="qTs", name=f"qTs{b}")
                kT_sb[b] = big.tile([128, FCH * S], BF16, tag="kTs", name=f"kTs{b}")
                v_sb[b] = big.tile(
                    [128, NKV * HPC * 65], BF16, tag="vs", name=f"vs{b}"
                )
                ones_ap = v_sb[b][:].rearrange("p (n c) -> p n c", c=65)[:, :, 64:65]
                nc.vector.memset(ones_ap, 1.0)  # ones col (idx 64) per 65-block

            qk_xt = {}

            def emit_qk_dma(b, which):
                # fp8 chunk-pair tiles [128, 2, S//2] for DoubleRow
                src_d = qT_d if which == "q" else kT_d
                src3 = src_d[:, :].rearrange("p (e r) -> p e r", e=NEC)
                tiles = {}
                for hf in range(2):
                    for ep in range(NEC // 2):
                        t = xin.tile(
                            [128, S], F8, tag="xin", name=f"x{which}{b}e{ep}h{hf}"
                        )
                        nc.sync.dma_start(
                            out=t[:].rearrange("p (j c) -> p j c", j=2),
                            in_=src3[
                                :,
                                2 * ep : 2 * ep + 2,
                                b * S + hf * (S // 2) : b * S + (hf + 1) * (S // 2),
                            ],
                        )
                        tiles[(ep, hf)] = t
                qk_xt[(b, which)] = tiles

            qk_done = {}

            def emit_qk_proj(b, which, rcs=None, fs=None):
                wsb, bsb = (wq_sb, bq_sb) if which == "q" else (wk_sb, bk_sb)
                dst = (qT_sb if which == "q" else kT_sb)[b]
                if (b, which) not in qk_xt:
                    emit_qk_dma(b, which)
                tiles = qk_xt[(b, which)]
                w3 = wsb[:].rearrange("p (e c) -> p e c", e=NEC)
                rcph = max(1, S // 2 // 512)
                if rcs is None:
                    rcs = range(S // 512)
                if fs is None:
                    fs = range(FCH)
                done = qk_done.setdefault((b, which), set())
                for rc in rcs:
                    hf, off = rc // rcph, (rc % rcph) * 512
                    for f in fs:
                        pt = pb_t[:, 0:512]
                        for ep in range(NEC // 2):
                            nc.tensor.matmul(
                                pt,
                                w3[:, 2 * ep : 2 * ep + 2, f * 128 : (f + 1) * 128],
                                tiles[(ep, hf)][:]
                                .rearrange("p (j c) -> p j c", j=2)[
                                    :, :, off : off + 512
                                ],
                                start=(ep == 0),
                                stop=(ep == NEC // 2 - 1),
                                perf_mode=DR,
                            )
                        nc.vector.tensor_scalar_add(
                            dst[:, f * S + rc * 512 : f * S + (rc + 1) * 512],
                            pt,
                            bsb[:, f : f + 1],
                        )
                        done.add((rc, f))
                if len(done) == (S // 512) * FCH:
                    del qk_xt[(b, which)]
                    del qk_done[(b, which)]

            def emit_v_dma(b):
                xt = {}
                for ec in range(NEC):
                    for hf in range(2):
                        t = xin.tile(
                            [128, S // 2], BF16, tag="xin", name=f"xv{b}e{ec}h{hf}"
                        )
                        nc.sync.dma_start(
                            out=t[:],
                            in_=vT_d[
                                ec * 128 : (ec + 1) * 128,
                                b * S + hf * (S // 2) : b * S + (hf + 1) * (S // 2),
                            ],
                        )
                        xt[(ec, hf)] = t
                v_xt[b] = xt

            v_done = {}

            def emit_v_proj(b, part=None, kvts=None):
                if b not in v_xt:
                    emit_v_dma(b)
                xt = v_xt[b]
                if kvts is None:
                    kvts = range(part * NKV // 2, (part + 1) * NKV // 2)
                done = v_done.setdefault(b, set())
                for kvt in kvts:
                    hf = kvt // (NKV // 2)
                    off = (kvt % (NKV // 2)) * 128
                    # pv = ob_t[256:512] (order-safe overlap with po1/tp,
                    # which are only live during outproj)
                    pv = ob_t[:, 256:512]
                    for ec in range(NEC):
                        nc.tensor.matmul(
                            pv,
                            xt[(ec, hf)][:, off : off + 128],
                            wv_sb[:, ec * DC : (ec + 1) * DC],
                            start=(ec == 0),
                            stop=(ec == NEC - 1),
                        )
                    for h in range(HPC):
                        c0 = (kvt * HPC + h) * 65
                        nc.vector.tensor_copy(
                            v_sb[b][:, c0 : c0 + 64], pv[:, h * 64 : (h + 1) * 64]
                        )
                    done.add(kvt)
                if len(done) == NKV:
                    del v_xt[b]
                    del v_done[b]

            def emit_proj_part(b, part):
                if part == 0:
                    emit_qk_proj(b, "q")
                elif part == 1:
                    emit_qk_proj(b, "k")
                else:
                    emit_v_proj(b, part - 2)

            apm = {}
            et_store = {}
            # score/exp groups per (b,h,qc): kvt runs [0:4),[4:6),[6:10),
            # [10:12),[12:16) on alternating spsA(2048)/spsB(1024) tiles
            SG = [(0, 4, "spsA"), (4, 2, "spsB"), (6, 4, "spsA"),
                  (10, 2, "spsB"), (12, 4, "spsA")]
            GIDX = [0, 0, 0, 0, 1, 1, 2, 2, 2, 2, 3, 3, 4, 4, 4, 4]
            GOFF = [0, 1, 2, 3, 0, 1, 0, 1, 2, 3, 0, 1, 0, 1, 2, 3]

            def emit_scores(b, h, qc):
                f, dh = h // 2, (h % 2) * 64
                qcol = qc * 512
                qTb, kTb = qT_sb[b], kT_sb[b]
                et = []
                for k0, klen, tag in SG:
                    w = klen * 512
                    sps = ps.tile([128, w], F32, tag=tag, bufs=1, name=tag)
                    for j in range(klen):
                        if ablate in ("pe0", "both") and j % 2 == 1:
                            continue  # timing ablation: halve scores matmuls
                        kvt = k0 + j
                        nc.tensor.matmul(
                            sps[:, j * 512 : (j + 1) * 512],
                            kTb[dh : dh + 64, f * S + kvt * 128 : f * S + (kvt + 1) * 128],
                            qTb[dh : dh + 64, f * S + qcol : f * S + qcol + 512],
                            start=True,
                            stop=True,
                        )
                    e_t = expp.tile([128, w], BF16, tag="expp", name="et")
                    if ablate in ("act0", "both"):
                        # timing ablation: halve ACT busy (upper half stale)
                        nc.scalar.activation(
                            e_t[:, 0 : w // 2], sps[:, 0 : w // 2], AF.Exp, scale=0.125
                        )
                    else:
                        nc.scalar.activation(e_t[:], sps[:], AF.Exp, scale=0.125)
                    et.append(e_t)
                et_store[h] = et

            def emit_av(b, h, qc):
                # AV position-major: per q-tile of 128, accumulate over 16 kv
                # tiles. Accumulators ap0/ap1 alternate in ob_t[0:130] so a
                # group never waits on the DVE drain of the previous group.
                hp, dh = h // 2, (h % 2) * 64
                vb = v_sb[b]
                et = et_store[h]
                for qt in range(4):
                    qoff = qt * 128
                    if h % 2 == 0:
                        apm[(hp, qt)] = apmp.tile(
                            [128, 128], BF16, tag="apm", name=f"apm{hp}_{qt}"
                        )
                    a0 = (qt % 2) * 65
                    ap = ob_t[:, a0 : a0 + 65]
                    nkv_eff = 4 if ablate == "av4" else NKV
                    for kvt in range(nkv_eff):
                        g, j = GIDX[kvt], GOFF[kvt]
                        c0 = (kvt * HPC + h) * 65
                        nc.tensor.matmul(
                            ap,
                            et[g][:, j * 512 + qoff : j * 512 + qoff + 128],
                            vb[:, c0 : c0 + 65],
                            start=(kvt == 0),
                            stop=(kvt == nkv_eff - 1),
                        )
                    rec = evp.tile([128, 1], F32, tag="rec", name="rec")
                    nc.vector.reciprocal(rec[:], ap[:, 64:65])
                    nc.vector.tensor_scalar_mul(
                        apm[(hp, qt)][:, dh : dh + 64], ap[:, 0:64], rec[:, 0:1]
                    )

            def emit_outproj(b, qc):
                for qt in range(4):
                    qcol = qc * 512 + qt * 128
                    afm = {}
                    for hp in range(FCH):
                        # tp0/tp1 live in ob_t[384:512] viewed as bf16
                        tp = ob_t[:, 384 + hp * 64 : 448 + hp * 64].bitcast(BF16)
                        nc.tensor.transpose(tp, apm[(hp, qt)][:], id_sb[:])
                        afm[hp] = evp.tile([128, 128], BF16, tag="afm", name="afm")
                        nc.vector.tensor_copy(afm[hp][:], tp)
                    ot = otp.tile([128, EMBED], BF16, tag="ot", name="ot")
                    for en in range(4):
                        po = ob_t[:, (en % 2) * 256 : (en % 2) * 256 + 256]
                        for hp in range(FCH):
                            nc.tensor.matmul(
                                po,
                                afm[hp][:],
                                wo_sb[:, hp * EMBED + en * 256 : hp * EMBED + en * 256 + 256],
                                start=(hp == 0),
                                stop=(hp == FCH - 1),
                            )
                        nc.vector.tensor_copy(ot[:, en * 256 : (en + 1) * 256], po)
                    nc.sync.dma_start(
                        out=out_d[b * S + qcol : b * S + qcol + 128, :], in_=ot[:]
                    )

            v_xt = {}

            def emit_iteration():
                # prologue: heads 0/1 only need feature-chunk 0 of kT/qT, so
                # emit k-proj f0 -> q-proj rc0 f0 -> first scores as early as
                # possible (first exp ~16us in), filling the rest of the
                # projections and the remaining weight DMAs under the first
                # exps. k-proj weights first: they gate the first scores.
                nc.sync.dma_start(out=wk_sb[:], in_=wk_d[:])
                nc.sync.dma_start(out=bk_sb[:], in_=bk_d[:])
                alloc_batch(0)
                emit_qk_dma(0, "k")
                nc.sync.dma_start(out=wq_sb[:], in_=wq_d[:])
                nc.sync.dma_start(out=bq_sb[:], in_=bq_d[:])
                emit_qk_dma(0, "q")
                emit_qk_proj(0, "k", fs=[0])
                emit_qk_proj(0, "q", rcs=[0], fs=[0])
                emit_scores(0, 0, 0)
                nc.sync.dma_start(out=wv_sb[:], in_=wv_d[:])
                emit_qk_proj(0, "k", fs=[1])
                emit_scores(0, 1, 0)
                emit_v_dma(0)
                emit_qk_proj(0, "q", rcs=[0], fs=[1])
                emit_qk_proj(0, "q", rcs=[1, 2, 3])
                nc.sync.dma_start(out=wo_sb[:], in_=wo_d[:])
                nc.sync.dma_start(out=id_sb[:], in_=id_d[:])
                emit_scores(0, 2, 0)
                emit_v_proj(0, 0)
                emit_v_proj(0, 1)
                # steady state: attention/outproj of b with proj work for b+1
                # sliced into ~1.7us pieces drained at several ladder points
                # per chunk, so no long PE block ever delays the next scores
                # (which would starve ACT, the critical engine)
                for b in range(BPC):
                    slices = []
                    if b + 1 < BPC:
                        alloc_batch(b + 1)
                        nb = b + 1
                        slices.append(lambda nb=nb: emit_qk_dma(nb, "q"))
                        for rc in range(S // 512):
                            for f in range(FCH):
                                slices.append(
                                    lambda nb=nb, rc=rc, f=f: emit_qk_proj(
                                        nb, "q", rcs=[rc], fs=[f]
                                    )
                                )
                        slices.append(lambda nb=nb: emit_qk_dma(nb, "k"))
                        for rc in range(S // 512):
                            for f in range(FCH):
                                slices.append(
                                    lambda nb=nb, rc=rc, f=f: emit_qk_proj(
                                        nb, "k", rcs=[rc], fs=[f]
                                    )
                                )
                        slices.append(lambda nb=nb: emit_v_dma(nb))
                        for k2 in range(NKV // 2):
                            slices.append(
                                lambda nb=nb, k2=k2: emit_v_proj(
                                    nb, kvts=[k2 * 2, k2 * 2 + 1]
                                )
                            )
                    sl = iter(slices)
                    left = len(slices)

                    def drain(n):
                        nonlocal left
                        for _ in range(n):
                            s = next(sl, None)
                            if s is None:
                                return
                            s()
                            left -= 1

                    for qc in range(NQC):
                        pre = b == 0 and qc == 0
                        # even share of remaining slices over remaining qcs
                        share = (left + (NQC - qc) - 1) // (NQC - qc)
                        if not pre and qc == 0:
                            emit_scores(b, 0, qc)
                        if not pre:
                            emit_scores(b, 1, qc)
                        emit_av(b, 0, qc)
                        if not pre:
                            emit_scores(b, 2, qc)
                        drain(share // 3)
                        emit_av(b, 1, qc)
                        emit_scores(b, 3, qc)
                        drain(share // 3)
                        emit_av(b, 2, qc)
                        emit_av(b, 3, qc)
                        drain(share - 2 * (share // 3))
                        if qc + 1 < NQC:
                            # peel next chunk's first scores ahead of outproj
                            # so ACT keeps a 2-group buffer at the boundary
                            emit_scores(b, 0, qc + 1)
                        emit_outproj(b, qc)

            for _ in range(iters):
                emit_iteration()

    nc.finalize()
    return nc


_NC_CACHE = {}


def get_nc(B=4, S=2048, lowering=False):
    key = (B, S, lowering)
    if key not in _NC_CACHE:
        _NC_CACHE[key] = build_nc(B, S, lowering)
    return _NC_CACHE[key]


def make_in_maps(value, key, query, Wv, bv, Wk, bk, Wq, bq, Wo, bo, B, S):
    ROWS = B * S
    bf = ml_dtypes.bfloat16
    f8 = ml_dtypes.float8_e4m3
    # fp8 chunk-major [128, NEC, ROWS] for q/k; bf16 feature-major for v
    qT8 = query.reshape(ROWS, EMBED).astype(f8).T.reshape(NEC, 128, ROWS)
    qT8 = np.ascontiguousarray(qT8.transpose(1, 0, 2))  # [128, NEC, ROWS]
    kT8 = key.reshape(ROWS, EMBED).astype(f8).T.reshape(NEC, 128, ROWS)
    kT8 = np.ascontiguousarray(kT8.transpose(1, 0, 2))
    vTh = np.ascontiguousarray(value.reshape(ROWS, EMBED).astype(bf).T)
    ident = np.eye(128, dtype=bf)
    in_maps = []
    for c in range(N_CORES):
        bg, hg = c // HPC, c % HPC
        rs = slice(bg * BPC * S, (bg + 1) * BPC * S)
        cs = slice(hg * DC, (hg + 1) * DC)

        def wchunks(W, dt):
            return np.ascontiguousarray(
                W[:, cs].astype(dt).reshape(NEC, 128, DC).transpose(1, 0, 2).reshape(128, NEC * DC)
            )

        in_maps.append(
            {
                "qT8": np.ascontiguousarray(qT8[:, :, rs]).reshape(128, -1),
                "kT8": np.ascontiguousarray(kT8[:, :, rs]).reshape(128, -1),
                "vT": np.ascontiguousarray(vTh[:, rs]),
                "wq8": wchunks(Wq, f8),
                "wk8": wchunks(Wk, f8),
                "wv": wchunks(Wv, bf),
                "bq": np.ascontiguousarray(
                    bq[cs].reshape(FCH, 128).T.astype(np.float32)
                ),
                "bk": np.ascontiguousarray(
                    bk[cs].reshape(FCH, 128).T.astype(np.float32)
                ),
                "wo": np.ascontiguousarray(
                    Wo[cs, :].astype(bf).reshape(FCH, 128, EMBED).transpose(1, 0, 2).reshape(128, FCH * EMBED)
                ),
                "ident": ident,
            }
        )
    return in_maps


def finish(results, Wv, bv, Wo, bo, B, S):
    const_row = (
        bv.astype(np.float32) @ Wo.astype(np.float32) + bo.astype(np.float32)
    )[None, :]
    out = np.empty((B * S, EMBED), np.float32)
    for bg in range(B // BPC):
        acc = results[bg * HPC]["out"].astype(np.float32)
        for hg in range(1, HPC):
            acc = acc + results[bg * HPC + hg]["out"].astype(np.float32)
        out[bg * BPC * S : (bg + 1) * BPC * S] = acc
    out += const_row
    return out.reshape(B, S, EMBED)


def kernel(value, key, query, Wv, bv, Wk, bk, Wq, bq, Wo, bo):
    B, S, _ = query.shape
    nc = get_nc(B, S)
    in_maps = make_in_maps(value, key, query, Wv, bv, Wk, bk, Wq, bq, Wo, bo, B, S)
    res = run_bass_kernel_spmd(nc, in_maps, list(range(N_CORES)))
    return finish(res.results, Wv, bv, Wo, bo, B, S)



# revision 25
# speedup vs baseline: 1.2947x; 1.2947x over previous
"""MultiHeadAttention TRN2 kernel — hybrid sharding: 2 batch-groups x 4
head-groups over 8 cores. Core c = (bg, hg) with bg = c//4, hg = c%4 owns
batches {2bg, 2bg+1} and heads {4hg..4hg+3} == feature columns
hg*256:(hg+1)*256 of Wq/Wk/Wv and rows hg*256:(hg+1)*256 of Wo.

vs pure head-parallel this halves the dominant DMA traffic (each core loads
x for 2 of 4 batches instead of all) while keeping per-core PE/ACT/DVE work
identical, and keeps >=2 local batches so projections of batch b+1 pipeline
under attention of batch b.

Device math (per core), all matmuls bf16 with f32 PSUM accumulation:
  qT/kT = (Wq_c^T x^T + bq_c)          feature-major [2x128, S] per batch
  v     = x^T^T Wv_c                   position-major, 65-wide blocks per
                                        (kv-tile, head); col 64 = ones so AV
                                        accumulates the softmax denominator
  scoresT[kv, q] = kT^T qT             per (b, h), exp via ACT with scale=1/8
  AV (position-major, full 128 PSUM partitions):
    ap[q, j] = sum_kv e[kv, q] v_aug[kv, j]
  attn_pm[q, d] = ap[q, d] / ap[q, 64]  per-partition scalar mul on DVE
  transpose attn_pm -> feature-major via PE identity transpose (f32)
  out_partial[q, e] = sum_hp attn_fm^T Wo_chunk   bf16 out, host sums the 4
                                        head-group partials per batch-group
"""

import sys

sys.path.insert(0, "/opt/trn_rl_repo")

import numpy as np
import ml_dtypes

import concourse.bass as bass
from concourse import bacc
import concourse.mybir as mybir
from concourse.tile import TileContext
from concourse.bass_utils import run_bass_kernel_spmd

BF16 = mybir.dt.bfloat16
F32 = mybir.dt.float32
F8 = mybir.dt.float8e4
AF = mybir.ActivationFunctionType
DR = mybir.MatmulPerfMode.DoubleRow

EMBED = 1024
HEADS = 16
HEAD_DIM = 64
N_CORES = 8
BPC = 2  # batches per core (batch-group size)
HPC = 4  # heads per core
DC = HPC * HEAD_DIM  # 256 feature columns per core
FCH = DC // 128  # feature chunks of 128
NEC = 8  # contraction chunks of 128 over EMBED


def build_nc(B=4, S=2048, lowering=False, iters=1, ablate=None):
    ROWS = BPC * S  # rows owned by this core's batch-group
    NQC = S // 512  # q chunks per (b, h)
    NKV = S // 128  # kv tiles per batch
    NKV2 = NKV // 2
    nc = bacc.Bacc("TRN2", target_bir_lowering=lowering)

    # q/k path in fp8e4 (chunk-major [128, NEC, ROWS] flattened) for
    # DoubleRow matmuls; v path stays bf16 feature-major.
    qT_d = nc.declare_dram_parameter("qT8", [128, NEC * ROWS], F8, isOutput=False)
    kT_d = nc.declare_dram_parameter("kT8", [128, NEC * ROWS], F8, isOutput=False)
    vT_d = nc.declare_dram_parameter("vT", [EMBED, ROWS], BF16, isOutput=False)
    wq_d = nc.declare_dram_parameter("wq8", [128, NEC * DC], F8, isOutput=False)
    wk_d = nc.declare_dram_parameter("wk8", [128, NEC * DC], F8, isOutput=False)
    wv_d = nc.declare_dram_parameter("wv", [128, NEC * DC], BF16, isOutput=False)
    bq_d = nc.declare_dram_parameter("bq", [128, FCH], F32, isOutput=False)
    bk_d = nc.declare_dram_parameter("bk", [128, FCH], F32, isOutput=False)
    wo_d = nc.declare_dram_parameter("wo", [128, FCH * EMBED], BF16, isOutput=False)
    id_d = nc.declare_dram_parameter("ident", [128, 128], BF16, isOutput=False)
    out_d = nc.declare_dram_parameter("out", [ROWS, EMBED], BF16, isOutput=True)

    with TileContext(nc) as tc:
        with (
            tc.tile_pool(name="const", bufs=1) as cpool,
            tc.tile_pool(name="big", bufs=2) as big,
            tc.tile_pool(name="xin", bufs=32) as xin,
            tc.tile_pool(name="expp", bufs=14) as expp,
            tc.tile_pool(name="apm", bufs=16) as apmp,
            tc.tile_pool(name="ev", bufs=8) as evp,
            tc.tile_pool(name="ot", bufs=4) as otp,
            tc.tile_pool(name="ps", bufs=1, space="PSUM") as ps,
        ):
            # --- weights / constants ---
            wq_sb = cpool.tile([128, NEC * DC], F8, tag="wq")
            wk_sb = cpool.tile([128, NEC * DC], F8, tag="wk")
            wv_sb = cpool.tile([128, NEC * DC], BF16, tag="wv")
            wo_sb = cpool.tile([128, FCH * EMBED], BF16, tag="wo")
            bq_sb = cpool.tile([128, FCH], F32, tag="bq")
            bk_sb = cpool.tile([128, FCH], F32, tag="bk")
            id_sb = cpool.tile([128, 128], BF16, tag="ident")

            # --- PSUM bank plan (8 banks total) ---
            # spsA (4 banks) / spsB (2 banks): alternating score tiles so one
            # exp instruction covers 2048/1024 columns (ACT per-instruction
            # overhead is ~270ns; fewer+bigger exps is the ACT win).
            # pb (1 bank): qk-proj psum pt [0:512].
            # ob (1 bank): po0 [0:256] / po1 [256:512] outproj ping-pong;
            #   ap0 [0:65], ap1 [65:130] AV accumulators (overlap po0 —
            #   order-safe: AV phase and outproj phase never overlap for the
            #   same region window); pv [256:384] v-proj psum (overlaps po1,
            #   order-safe for the same reason); tp0/tp1 [384:448]/[448:512]
            #   bitcast-bf16 transpose outputs (overlap po1, drained by afm
            #   copies before po1's writes in program order).
            pb_t = ps.tile([128, 512], F32, tag="pb", bufs=1, name="pb")
            ob_t = ps.tile([128, 512], F32, tag="ob", bufs=1, name="ob")

            qT_sb, kT_sb, v_sb = {}, {}, {}

            def alloc_batch(b):
                qT_sb[b] = big.tile([128, FCH * S], BF16, tag="qTs", name=f"qTs{b}")
                kT_sb[b] = big.tile([128, FCH * S], BF16, tag="kTs", name=f"kTs{b}")
                v_sb[b] = big.tile(
                    [128, NKV * HPC * 65], BF16, tag="vs", name=f"vs{b}"
                )
                ones_ap = v_sb[b][:].rearrange("p (n c) -> p n c", c=65)[:, :, 64:65]
                nc.vector.memset(ones_ap, 1.0)  # ones col (idx 64) per 65-block

            qk_xt = {}

            def emit_qk_dma(b, which):
                # fp8 chunk-pair tiles [128, 2, S//2] for DoubleRow
                src_d = qT_d if which == "q" else kT_d
                src3 = src_d[:, :].rearrange("p (e r) -> p e r", e=NEC)
                tiles = {}
                for hf in range(2):
                    for ep in range(NEC // 2):
                        t = xin.tile(
                            [128, S], F8, tag="xin", name=f"x{which}{b}e{ep}h{hf}"
                        )
                        nc.sync.dma_start(
                            out=t[:].rearrange("p (j c) -> p j c", j=2),
                            in_=src3[
                                :,
                                2 * ep : 2 * ep + 2,
                                b * S + hf * (S // 2) : b * S + (hf + 1) * (S // 2),
                            ],
                        )
                        tiles[(ep, hf)] = t
                qk_xt[(b, which)] = tiles

            qk_done = {}

            def emit_qk_proj(b, which, rcs=None, fs=None):
                wsb, bsb = (wq_sb, bq_sb) if which == "q" else (wk_sb, bk_sb)
                dst = (qT_sb if which == "q" else kT_sb)[b]
                if (b, which) not in qk_xt:
                    emit_qk_dma(b, which)
                tiles = qk_xt[(b, which)]
                w3 = wsb[:].rearrange("p (e c) -> p e c", e=NEC)
                rcph = max(1, S // 2 // 512)
                if rcs is None:
                    rcs = range(S // 512)
                if fs is None:
                    fs = range(FCH)
                done = qk_done.setdefault((b, which), set())
                for rc in rcs:
                    hf, off = rc // rcph, (rc % rcph) * 512
                    for f in fs:
                        pt = pb_t[:, 0:512]
                        for ep in range(NEC // 2):
                            nc.tensor.matmul(
                                pt,
                                w3[:, 2 * ep : 2 * ep + 2, f * 128 : (f + 1) * 128],
                                tiles[(ep, hf)][:]
                                .rearrange("p (j c) -> p j c", j=2)[
                                    :, :, off : off + 512
                                ],
                                start=(ep == 0),
                                stop=(ep == NEC // 2 - 1),
                                perf_mode=DR,
                            )
                        nc.vector.tensor_scalar_add(
                            dst[:, f * S + rc * 512 : f * S + (rc + 1) * 512],
                            pt,
                            bsb[:, f : f + 1],
                        )
                        done.add((rc, f))
                if len(done) == (S // 512) * FCH:
                    del qk_xt[(b, which)]
                    del qk_done[(b, which)]

            def emit_v_dma(b):
                xt = {}
                for ec in range(NEC):
                    for hf in range(2):
                        t = xin.tile(
                            [128, S // 2], BF16, tag="xin", name=f"xv{b}e{ec}h{hf}"
                        )
                        nc.sync.dma_start(
                            out=t[:],
                            in_=vT_d[
                                ec * 128 : (ec + 1) * 128,
                                b * S + hf * (S // 2) : b * S + (hf + 1) * (S // 2),
                            ],
                        )
                        xt[(ec, hf)] = t
                v_xt[b] = xt

            v_done = {}

            def emit_v_proj(b, part=None, kvts=None):
                if b not in v_xt:
                    emit_v_dma(b)
                xt = v_xt[b]
                if kvts is None:
                    kvts = range(part * NKV // 2, (part + 1) * NKV // 2)
                done = v_done.setdefault(b, set())
                for kvt in kvts:
                    hf = kvt // (NKV // 2)
                    off = (kvt % (NKV // 2)) * 128
                    # pv = ob_t[256:512] (order-safe overlap with po1/tp,
                    # which are only live during outproj)
                    pv = ob_t[:, 256:512]
                    for ec in range(NEC):
                        nc.tensor.matmul(
                            pv,
                            xt[(ec, hf)][:, off : off + 128],
                            wv_sb[:, ec * DC : (ec + 1) * DC],
                            start=(ec == 0),
                            stop=(ec == NEC - 1),
                        )
                    for h in range(HPC):
                        c0 = (kvt * HPC + h) * 65
                        nc.vector.tensor_copy(
                            v_sb[b][:, c0 : c0 + 64], pv[:, h * 64 : (h + 1) * 64]
                        )
                    done.add(kvt)
                if len(done) == NKV:
                    del v_xt[b]
                    del v_done[b]

            def emit_proj_part(b, part):
                if part == 0:
                    emit_qk_proj(b, "q")
                elif part == 1:
                    emit_qk_proj(b, "k")
                else:
                    emit_v_proj(b, part - 2)

            apm = {}
            et_store = {}
            # score/exp groups per (b,h,qc): five 3-kvt groups + one 1-kvt
            # group on alternating 3-bank spsA/spsB tiles (double-buffered:
            # PE fills one slot while ACT exps the other)
            SG = [(0, 3, "spsA"), (3, 3, "spsB"), (6, 3, "spsA"),
                  (9, 3, "spsB"), (12, 3, "spsA"), (15, 1, "spsB")]
            GIDX = [0, 0, 0, 1, 1, 1, 2, 2, 2, 3, 3, 3, 4, 4, 4, 5]
            GOFF = [0, 1, 2, 0, 1, 2, 0, 1, 2, 0, 1, 2, 0, 1, 2, 0]

            def emit_scores(b, h, qc):
                f, dh = h // 2, (h % 2) * 64
                qcol = qc * 512
                qTb, kTb = qT_sb[b], kT_sb[b]
                et = []
                for k0, klen, tag in SG:
                    w = klen * 512
                    sps = ps.tile([128, w], F32, tag=tag, bufs=1, name=tag)
                    for j in range(klen):
                        if ablate in ("pe0", "both") and j % 2 == 1:
                            continue  # timing ablation: halve scores matmuls
                        kvt = k0 + j
                        nc.tensor.matmul(
                            sps[:, j * 512 : (j + 1) * 512],
                            kTb[dh : dh + 64, f * S + kvt * 128 : f * S + (kvt + 1) * 128],
                            qTb[dh : dh + 64, f * S + qcol : f * S + qcol + 512],
                            start=True,
                            stop=True,
                        )
                    e_t = expp.tile([128, w], BF16, tag="expp", name="et")
                    if ablate in ("act0", "both"):
                        # timing ablation: halve ACT busy (upper half stale)
                        nc.scalar.activation(
                            e_t[:, 0 : w // 2], sps[:, 0 : w // 2], AF.Exp, scale=0.125
                        )
                    else:
                        nc.scalar.activation(e_t[:], sps[:], AF.Exp, scale=0.125)
                    et.append(e_t)
                et_store[h] = et

            def emit_av(b, h, qc):
                # AV position-major: per q-tile of 128, accumulate over 16 kv
                # tiles. Accumulators ap0/ap1 alternate in ob_t[0:130] so a
                # group never waits on the DVE drain of the previous group.
                hp, dh = h // 2, (h % 2) * 64
                vb = v_sb[b]
                et = et_store[h]
                for qt in range(4):
                    qoff = qt * 128
                    if h % 2 == 0:
                        apm[(hp, qt)] = apmp.tile(
                            [128, 128], BF16, tag="apm", name=f"apm{hp}_{qt}"
                        )
                    a0 = (qt % 2) * 65
                    ap = ob_t[:, a0 : a0 + 65]
                    nkv_eff = 4 if ablate == "av4" else NKV
                    for kvt in range(nkv_eff):
                        g, j = GIDX[kvt], GOFF[kvt]
                        c0 = (kvt * HPC + h) * 65
                        nc.tensor.matmul(
                            ap,
                            et[g][:, j * 512 + qoff : j * 512 + qoff + 128],
                            vb[:, c0 : c0 + 65],
                            start=(kvt == 0),
                            stop=(kvt == nkv_eff - 1),
                        )
                    rec = evp.tile([128, 1], F32, tag="rec", name="rec")
                    nc.vector.reciprocal(rec[:], ap[:, 64:65])
                    nc.vector.tensor_scalar_mul(
                        apm[(hp, qt)][:, dh : dh + 64], ap[:, 0:64], rec[:, 0:1]
                    )

            def emit_outproj(b, qc):
                for qt in range(4):
                    qcol = qc * 512 + qt * 128
                    afm = {}
                    for hp in range(FCH):
                        # tp0/tp1 live in ob_t[384:512] viewed as bf16
                        tp = ob_t[:, 384 + hp * 64 : 448 + hp * 64].bitcast(BF16)
                        nc.tensor.transpose(tp, apm[(hp, qt)][:], id_sb[:])
                        afm[hp] = evp.tile([128, 128], BF16, tag="afm", name="afm")
                        nc.vector.tensor_copy(afm[hp][:], tp)
                    ot = otp.tile([128, EMBED], BF16, tag="ot", name="ot")
                    for en in range(4):
                        po = ob_t[:, (en % 2) * 256 : (en % 2) * 256 + 256]
                        for hp in range(FCH):
                            nc.tensor.matmul(
                                po,
                                afm[hp][:],
                                wo_sb[:, hp * EMBED + en * 256 : hp * EMBED + en * 256 + 256],
                                start=(hp == 0),
                                stop=(hp == FCH - 1),
                            )
                        nc.vector.tensor_copy(ot[:, en * 256 : (en + 1) * 256], po)
                    nc.sync.dma_start(
                        out=out_d[b * S + qcol : b * S + qcol + 128, :], in_=ot[:]
                    )

            v_xt = {}

            def emit_iteration():
                # prologue: heads 0/1 only need feature-chunk 0 of kT/qT, so
                # emit k-proj f0 -> q-proj rc0 f0 -> first scores as early as
                # possible (first exp ~16us in), filling the rest of the
                # projections and the remaining weight DMAs under the first
                # exps. k-proj weights first: they gate the first scores.
                nc.sync.dma_start(out=wk_sb[:], in_=wk_d[:])
                nc.sync.dma_start(out=bk_sb[:], in_=bk_d[:])
                alloc_batch(0)
                emit_qk_dma(0, "k")
                nc.sync.dma_start(out=wq_sb[:], in_=wq_d[:])
                nc.sync.dma_start(out=bq_sb[:], in_=bq_d[:])
                emit_qk_dma(0, "q")
                emit_qk_proj(0, "k", fs=[0])
                emit_qk_proj(0, "q", rcs=[0], fs=[0])
                emit_scores(0, 0, 0)
                nc.sync.dma_start(out=wv_sb[:], in_=wv_d[:])
                emit_qk_proj(0, "k", fs=[1])
                emit_scores(0, 1, 0)
                emit_v_dma(0)
                emit_qk_proj(0, "q", rcs=[0], fs=[1])
                emit_qk_proj(0, "q", rcs=[1, 2, 3])
                nc.sync.dma_start(out=wo_sb[:], in_=wo_d[:])
                nc.sync.dma_start(out=id_sb[:], in_=id_d[:])
                emit_scores(0, 2, 0)
                emit_v_proj(0, 0)
                emit_v_proj(0, 1)
                # steady state: attention/outproj of b with proj work for b+1
                # sliced into ~1.7us pieces drained at several ladder points
                # per chunk, so no long PE block ever delays the next scores
                # (which would starve ACT, the critical engine)
                for b in range(BPC):
                    slices = []
                    if b + 1 < BPC:
                        alloc_batch(b + 1)
                        nb = b + 1
                        slices.append(lambda nb=nb: emit_qk_dma(nb, "q"))
                        for rc in range(S // 512):
                            for f in range(FCH):
                                slices.append(
                                    lambda nb=nb, rc=rc, f=f: emit_qk_proj(
                                        nb, "q", rcs=[rc], fs=[f]
                                    )
                                )
                        slices.append(lambda nb=nb: emit_qk_dma(nb, "k"))
                        for rc in range(S // 512):
                            for f in range(FCH):
                                slices.append(
                                    lambda nb=nb, rc=rc, f=f: emit_qk_proj(
                                        nb, "k", rcs=[rc], fs=[f]
                                    )
                                )
                        slices.append(lambda nb=nb: emit_v_dma(nb))
                        for k2 in range(NKV // 2):
                            slices.append(
                                lambda nb=nb, k2=k2: emit_v_proj(
                                    nb, kvts=[k2 * 2, k2 * 2 + 1]
                                )
                            )
                    sl = iter(slices)
                    left = len(slices)

                    def drain(n):
                        nonlocal left
                        for _ in range(n):
                            s = next(sl, None)
                            if s is None:
                                return
                            s()
                            left -= 1

                    for qc in range(NQC):
                        pre = b == 0 and qc == 0
                        # even share of remaining slices over remaining qcs
                        share = (left + (NQC - qc) - 1) // (NQC - qc)
                        if not pre and qc == 0:
                            emit_scores(b, 0, qc)
                        if not pre:
                            emit_scores(b, 1, qc)
                        emit_av(b, 0, qc)
                        if not pre:
                            emit_scores(b, 2, qc)
                        drain(share // 3)
                        emit_av(b, 1, qc)
                        emit_scores(b, 3, qc)
                        drain(share // 3)
                        emit_av(b, 2, qc)
                        emit_av(b, 3, qc)
                        drain(share - 2 * (share // 3))
                        if qc + 1 < NQC:
                            # peel next chunk's first scores ahead of outproj
                            # so ACT keeps a 2-group buffer at the boundary
                            emit_scores(b, 0, qc + 1)
                        emit_outproj(b, qc)

            for _ in range(iters):
                emit_iteration()

    nc.finalize()
    return nc


_NC_CACHE = {}


def get_nc(B=4, S=2048, lowering=False):
    key = (B, S, lowering)
    if key not in _NC_CACHE:
        _NC_CACHE[key] = build_nc(B, S, lowering)
    return _NC_CACHE[key]


def make_in_maps(value, key, query, Wv, bv, Wk, bk, Wq, bq, Wo, bo, B, S):
    ROWS = B * S
    bf = ml_dtypes.bfloat16
    f8 = ml_dtypes.float8_e4m3
    # fp8 chunk-major [128, NEC, ROWS] for q/k; bf16 feature-major for v
    qT8 = query.reshape(ROWS, EMBED).astype(f8).T.reshape(NEC, 128, ROWS)
    qT8 = np.ascontiguousarray(qT8.transpose(1, 0, 2))  # [128, NEC, ROWS]
    kT8 = key.reshape(ROWS, EMBED).astype(f8).T.reshape(NEC, 128, ROWS)
    kT8 = np.ascontiguousarray(kT8.transpose(1, 0, 2))
    vTh = np.ascontiguousarray(value.reshape(ROWS, EMBED).astype(bf).T)
    ident = np.eye(128, dtype=bf)
    in_maps = []
    for c in range(N_CORES):
        bg, hg = c // HPC, c % HPC
        rs = slice(bg * BPC * S, (bg + 1) * BPC * S)
        cs = slice(hg * DC, (hg + 1) * DC)

        def wchunks(W, dt):
            return np.ascontiguousarray(
                W[:, cs].astype(dt).reshape(NEC, 128, DC).transpose(1, 0, 2).reshape(128, NEC * DC)
            )

        in_maps.append(
            {
                "qT8": np.ascontiguousarray(qT8[:, :, rs]).reshape(128, -1),
                "kT8": np.ascontiguousarray(kT8[:, :, rs]).reshape(128, -1),
                "vT": np.ascontiguousarray(vTh[:, rs]),
                "wq8": wchunks(Wq, f8),
                "wk8": wchunks(Wk, f8),
                "wv": wchunks(Wv, bf),
                "bq": np.ascontiguousarray(
                    bq[cs].reshape(FCH, 128).T.astype(np.float32)
                ),
                "bk": np.ascontiguousarray(
                    bk[cs].reshape(FCH, 128).T.astype(np.float32)
                ),
                "wo": np.ascontiguousarray(
                    Wo[cs, :].astype(bf).reshape(FCH, 128, EMBED).transpose(1, 0, 2).reshape(128, FCH * EMBED)
                ),
                "ident": ident,
            }
        )
    return in_maps


def finish(results, Wv, bv, Wo, bo, B, S):
    const_row = (
        bv.astype(np.float32) @ Wo.astype(np.float32) + bo.astype(np.float32)
    )[None, :]
    out = np.empty((B * S, EMBED), np.float32)
    for bg in range(B // BPC):
        acc = results[bg * HPC]["out"].astype(np.float32)
        for hg in range(1, HPC):
            acc = acc + results[bg * HPC + hg]["out"].astype(np.float32)
        out[bg * BPC * S : (bg + 1) * BPC * S] = acc
    out += const_row
    return out.reshape(B, S, EMBED)


def kernel(value, key, query, Wv, bv, Wk, bk, Wq, bq, Wo, bo):
    B, S, _ = query.shape
    nc = get_nc(B, S)
    in_maps = make_in_maps(value, key, query, Wv, bv, Wk, bk, Wq, bq, Wo, bo, B, S)
    res = run_bass_kernel_spmd(nc, in_maps, list(range(N_CORES)))
    return finish(res.results, Wv, bv, Wo, bo, B, S)



# revision 33
# speedup vs baseline: 1.5455x; 1.1937x over previous
"""MultiHeadAttention TRN2 kernel — hybrid sharding: 2 batch-groups x 4
head-groups over 8 cores. Core c = (bg, hg) with bg = c//4, hg = c%4 owns
batches {2bg, 2bg+1} and heads {4hg..4hg+3} == feature columns
hg*256:(hg+1)*256 of Wq/Wk/Wv and rows hg*256:(hg+1)*256 of Wo.

vs pure head-parallel this halves the dominant DMA traffic (each core loads
x for 2 of 4 batches instead of all) while keeping per-core PE/ACT/DVE work
identical, and keeps >=2 local batches so projections of batch b+1 pipeline
under attention of batch b.

Device math (per core), all matmuls bf16 with f32 PSUM accumulation:
  qT/kT = (Wq_c^T x^T + bq_c)          feature-major [2x128, S] per batch
  v     = x^T^T Wv_c                   position-major, 65-wide blocks per
                                        (kv-tile, head); col 64 = ones so AV
                                        accumulates the softmax denominator
  scoresT[kv, q] = kT^T qT             per (b, h), exp via ACT with scale=1/8
  AV (position-major, full 128 PSUM partitions):
    ap[q, j] = sum_kv e[kv, q] v_aug[kv, j]
  attn_pm[q, d] = ap[q, d] / ap[q, 64]  per-partition scalar mul on DVE
  transpose attn_pm -> feature-major via PE identity transpose (f32)
  out_partial[q, e] = sum_hp attn_fm^T Wo_chunk   bf16 out, host sums the 4
                                        head-group partials per batch-group
"""

import sys

sys.path.insert(0, "/opt/trn_rl_repo")

import numpy as np
import ml_dtypes

import concourse.bass as bass
from concourse import bacc
import concourse.mybir as mybir
from concourse.tile import TileContext
from concourse.bass_utils import run_bass_kernel_spmd

BF16 = mybir.dt.bfloat16
F32 = mybir.dt.float32
F8 = mybir.dt.float8e4
AF = mybir.ActivationFunctionType
DR = mybir.MatmulPerfMode.DoubleRow

EMBED = 1024
HEADS = 16
HEAD_DIM = 64
N_CORES = 8
BPC = 2  # batches per core (batch-group size)
HPC = 4  # heads per core
DC = HPC * HEAD_DIM  # 256 feature columns per core
FCH = DC // 128  # feature chunks of 128
NEC = 8  # contraction chunks of 128 over EMBED


def build_nc(B=4, S=2048, lowering=False, iters=1, ablate=None):
    ROWS = BPC * S  # rows owned by this core's batch-group
    NQC = S // 512  # q chunks per (b, h)
    NKV = S // 128  # kv tiles per batch
    NKV2 = NKV // 2
    nc = bacc.Bacc("TRN2", target_bir_lowering=lowering)

    # q/k path in fp8e4 (chunk-major [128, NEC, ROWS] flattened) for
    # DoubleRow matmuls; v path stays bf16 feature-major.
    qT_d = nc.declare_dram_parameter("qT8", [128, NEC * ROWS], F8, isOutput=False)
    kT_d = nc.declare_dram_parameter("kT8", [128, NEC * ROWS], F8, isOutput=False)
    vT_d = nc.declare_dram_parameter("vT", [EMBED, ROWS], BF16, isOutput=False)
    wq_d = nc.declare_dram_parameter("wq8", [128, NEC * DC], F8, isOutput=False)
    wk_d = nc.declare_dram_parameter("wk8", [128, NEC * DC], F8, isOutput=False)
    wv_d = nc.declare_dram_parameter("wv", [128, NEC * DC], BF16, isOutput=False)
    bq_d = nc.declare_dram_parameter("bq", [128, FCH], F32, isOutput=False)
    bk_d = nc.declare_dram_parameter("bk", [128, FCH], F32, isOutput=False)
    wo_d = nc.declare_dram_parameter("wo", [128, FCH * EMBED], BF16, isOutput=False)
    id_d = nc.declare_dram_parameter("ident", [128, 128], BF16, isOutput=False)
    out_d = nc.declare_dram_parameter("out", [ROWS, EMBED], BF16, isOutput=True)

    with TileContext(nc) as tc:
        with (
            tc.tile_pool(name="const", bufs=1) as cpool,
            tc.tile_pool(name="big", bufs=2) as big,
            tc.tile_pool(name="xin", bufs=32) as xin,
            tc.tile_pool(name="expp", bufs=14) as expp,
            tc.tile_pool(name="apm", bufs=16) as apmp,
            tc.tile_pool(name="ev", bufs=8) as evp,
            tc.tile_pool(name="ot", bufs=4) as otp,
            tc.tile_pool(name="ps", bufs=1, space="PSUM") as ps,
        ):
            # --- weights / constants ---
            wq_sb = cpool.tile([128, NEC * DC], F8, tag="wq")
            wk_sb = cpool.tile([128, NEC * DC], F8, tag="wk")
            wv_sb = cpool.tile([128, NEC * DC], BF16, tag="wv")
            wo_sb = cpool.tile([128, FCH * EMBED], BF16, tag="wo")
            bq_sb = cpool.tile([128, FCH], F32, tag="bq")
            bk_sb = cpool.tile([128, FCH], F32, tag="bk")
            id_sb = cpool.tile([128, 128], BF16, tag="ident")

            # --- PSUM bank plan (8 banks; deps are BANK-granular!) ---
            # "sps" ring: 2 x [128,1536] f32 (3 banks each) so one exp
            # instruction covers 1536 columns (ACT per-instruction overhead
            # is ~270ns; fewer+bigger exps is the ACT win), double-buffered
            # so PE fills one slot while ACT drains the other.
            # "mr" ring: 2 x 1 bank rotated by ALL small psum users (AV
            # accumulators, proj/outproj outputs) in program order; each
            # user's WAR lands on the drain of the use two allocations back.
            qT_sb, kT_sb, v_sb = {}, {}, {}

            def alloc_batch(b):
                qT_sb[b] = big.tile([128, FCH * S], BF16, tag="qTs", name=f"qTs{b}")
                kT_sb[b] = big.tile([128, FCH * S], BF16, tag="kTs", name=f"kTs{b}")
                v_sb[b] = big.tile(
                    [128, NKV * HPC * 65], BF16, tag="vs", name=f"vs{b}"
                )
                ones_ap = v_sb[b][:].rearrange("p (n c) -> p n c", c=65)[:, :, 64:65]
                nc.vector.memset(ones_ap, 1.0)  # ones col (idx 64) per 65-block

            qk_xt = {}

            def emit_qk_dma(b, which):
                # fp8 chunk-pair tiles [128, 2, S//2] for DoubleRow
                src_d = qT_d if which == "q" else kT_d
                src3 = src_d[:, :].rearrange("p (e r) -> p e r", e=NEC)
                tiles = {}
                for hf in range(2):
                    for ep in range(NEC // 2):
                        t = xin.tile(
                            [128, S], F8, tag="xin", name=f"x{which}{b}e{ep}h{hf}"
                        )
                        nc.sync.dma_start(
                            out=t[:].rearrange("p (j c) -> p j c", j=2),
                            in_=src3[
                                :,
                                2 * ep : 2 * ep + 2,
                                b * S + hf * (S // 2) : b * S + (hf + 1) * (S // 2),
                            ],
                        )
                        tiles[(ep, hf)] = t
                qk_xt[(b, which)] = tiles

            qk_done = {}

            def emit_qk_proj(b, which, rcs=None, fs=None):
                wsb, bsb = (wq_sb, bq_sb) if which == "q" else (wk_sb, bk_sb)
                dst = (qT_sb if which == "q" else kT_sb)[b]
                if (b, which) not in qk_xt:
                    emit_qk_dma(b, which)
                tiles = qk_xt[(b, which)]
                w3 = wsb[:].rearrange("p (e c) -> p e c", e=NEC)
                rcph = max(1, S // 2 // 512)
                if rcs is None:
                    rcs = range(S // 512)
                if fs is None:
                    fs = range(FCH)
                done = qk_done.setdefault((b, which), set())
                for rc in rcs:
                    hf, off = rc // rcph, (rc % rcph) * 512
                    for f in fs:
                        ptt = ps.tile([128, 512], F32, tag="mr", bufs=2, name="pt")
                        pt = ptt[:]
                        for ep in range(NEC // 2):
                            nc.tensor.matmul(
                                pt,
                                w3[:, 2 * ep : 2 * ep + 2, f * 128 : (f + 1) * 128],
                                tiles[(ep, hf)][:]
                                .rearrange("p (j c) -> p j c", j=2)[
                                    :, :, off : off + 512
                                ],
                                start=(ep == 0),
                                stop=(ep == NEC // 2 - 1),
                                perf_mode=DR,
                            )
                        nc.vector.tensor_scalar_add(
                            dst[:, f * S + rc * 512 : f * S + (rc + 1) * 512],
                            pt,
                            bsb[:, f : f + 1],
                        )
                        done.add((rc, f))
                if len(done) == (S // 512) * FCH:
                    del qk_xt[(b, which)]
                    del qk_done[(b, which)]

            def emit_v_dma(b):
                xt = {}
                for ec in range(NEC):
                    for hf in range(2):
                        t = xin.tile(
                            [128, S // 2], BF16, tag="xin", name=f"xv{b}e{ec}h{hf}"
                        )
                        nc.sync.dma_start(
                            out=t[:],
                            in_=vT_d[
                                ec * 128 : (ec + 1) * 128,
                                b * S + hf * (S // 2) : b * S + (hf + 1) * (S // 2),
                            ],
                        )
                        xt[(ec, hf)] = t
                v_xt[b] = xt

            v_done = {}

            def emit_v_proj(b, part=None, kvts=None):
                if b not in v_xt:
                    emit_v_dma(b)
                xt = v_xt[b]
                if kvts is None:
                    kvts = range(part * NKV // 2, (part + 1) * NKV // 2)
                done = v_done.setdefault(b, set())
                for kvt in kvts:
                    hf = kvt // (NKV // 2)
                    off = (kvt % (NKV // 2)) * 128
                    pvt = ps.tile([128, 256], F32, tag="mr", bufs=2, name="pv")
                    pv = pvt[:]
                    for ec in range(NEC):
                        nc.tensor.matmul(
                            pv,
                            xt[(ec, hf)][:, off : off + 128],
                            wv_sb[:, ec * DC : (ec + 1) * DC],
                            start=(ec == 0),
                            stop=(ec == NEC - 1),
                        )
                    for h in range(HPC):
                        c0 = (kvt * HPC + h) * 65
                        nc.vector.tensor_copy(
                            v_sb[b][:, c0 : c0 + 64], pv[:, h * 64 : (h + 1) * 64]
                        )
                    done.add(kvt)
                if len(done) == NKV:
                    del v_xt[b]
                    del v_done[b]

            def emit_proj_part(b, part):
                if part == 0:
                    emit_qk_proj(b, "q")
                elif part == 1:
                    emit_qk_proj(b, "k")
                else:
                    emit_v_proj(b, part - 2)

            apm = {}
            et_store = {}
            # score/exp groups per (b,h,qc): five 3-kvt groups + one 1-kvt
            # group on the double-buffered 3-bank "sps" ring
            SG = [(0, 3), (3, 3), (6, 3), (9, 3), (12, 3), (15, 1)]
            GIDX = [0, 0, 0, 1, 1, 1, 2, 2, 2, 3, 3, 3, 4, 4, 4, 5]
            GOFF = [0, 1, 2, 0, 1, 2, 0, 1, 2, 0, 1, 2, 0, 1, 2, 0]

            def emit_scores(b, h, qc):
                f, dh = h // 2, (h % 2) * 64
                qcol = qc * 512
                qTb, kTb = qT_sb[b], kT_sb[b]
                et = []
                for k0, klen in SG:
                    w = klen * 512
                    sps = ps.tile([128, 1536], F32, tag="sps", bufs=2, name="sps")
                    for j in range(klen):
                        if ablate in ("pe0", "both") and j % 2 == 1:
                            continue  # timing ablation: halve scores matmuls
                        kvt = k0 + j
                        nc.tensor.matmul(
                            sps[:, j * 512 : (j + 1) * 512],
                            kTb[dh : dh + 64, f * S + kvt * 128 : f * S + (kvt + 1) * 128],
                            qTb[dh : dh + 64, f * S + qcol : f * S + qcol + 512],
                            start=True,
                            stop=True,
                        )
                    e_t = expp.tile([128, w], BF16, tag="expp", name="et")
                    if ablate in ("act0", "both"):
                        # timing ablation: halve ACT busy (upper half stale)
                        nc.scalar.activation(
                            e_t[:, 0 : w // 2], sps[:, 0 : w // 2], AF.Exp, scale=0.125
                        )
                    else:
                        nc.scalar.activation(e_t[:], sps[:, 0:w], AF.Exp, scale=0.125)
                    et.append(e_t)
                et_store[h] = et

            def emit_av(b, h, qc):
                # AV position-major: per q-tile of 128, accumulate over 16 kv
                # tiles. Accumulators ap0/ap1 alternate in ob_t[0:130] so a
                # group never waits on the DVE drain of the previous group.
                hp, dh = h // 2, (h % 2) * 64
                vb = v_sb[b]
                et = et_store[h]
                for qt in range(4):
                    qoff = qt * 128
                    if h % 2 == 0:
                        apm[(hp, qt)] = apmp.tile(
                            [128, 128], BF16, tag="apm", name=f"apm{hp}_{qt}"
                        )
                    ap = ps.tile([128, 65], F32, tag="mr", bufs=2, name="aps")
                    nkv_eff = 4 if ablate == "av4" else NKV
                    for kvt in range(nkv_eff):
                        g, j = GIDX[kvt], GOFF[kvt]
                        c0 = (kvt * HPC + h) * 65
                        nc.tensor.matmul(
                            ap[:],
                            et[g][:, j * 512 + qoff : j * 512 + qoff + 128],
                            vb[:, c0 : c0 + 65],
                            start=(kvt == 0),
                            stop=(kvt == nkv_eff - 1),
                        )
                    rec = evp.tile([128, 1], F32, tag="rec", name="rec")
                    nc.vector.reciprocal(rec[:], ap[:, 64:65])
                    nc.vector.tensor_scalar_mul(
                        apm[(hp, qt)][:, dh : dh + 64], ap[:, 0:64], rec[:, 0:1]
                    )

            def emit_outproj(b, qc):
                for qt in range(4):
                    qcol = qc * 512 + qt * 128
                    afm = {}
                    for hp in range(FCH):
                        tp = ps.tile([128, 128], BF16, tag="mr", bufs=2, name="tp")
                        nc.tensor.transpose(tp[:], apm[(hp, qt)][:], id_sb[:])
                        afm[hp] = evp.tile([128, 128], BF16, tag="afm", name="afm")
                        nc.vector.tensor_copy(afm[hp][:], tp[:])
                    ot = otp.tile([128, EMBED], BF16, tag="ot", name="ot")
                    for en in range(2):
                        po = ps.tile([128, 512], F32, tag="mr", bufs=2, name="po")
                        for hp in range(FCH):
                            nc.tensor.matmul(
                                po[:],
                                afm[hp][:],
                                wo_sb[:, hp * EMBED + en * 512 : hp * EMBED + en * 512 + 512],
                                start=(hp == 0),
                                stop=(hp == FCH - 1),
                            )
                        nc.vector.tensor_copy(ot[:, en * 512 : (en + 1) * 512], po[:])
                    nc.sync.dma_start(
                        out=out_d[b * S + qcol : b * S + qcol + 128, :], in_=ot[:]
                    )

            v_xt = {}

            def emit_iteration():
                # prologue: heads 0/1 only need feature-chunk 0 of kT/qT, so
                # emit k-proj f0 -> q-proj rc0 f0 -> first scores as early as
                # possible (first exp ~16us in), filling the rest of the
                # projections and the remaining weight DMAs under the first
                # exps. k-proj weights first: they gate the first scores.
                nc.sync.dma_start(out=wk_sb[:], in_=wk_d[:])
                nc.sync.dma_start(out=bk_sb[:], in_=bk_d[:])
                alloc_batch(0)
                emit_qk_dma(0, "k")
                nc.sync.dma_start(out=wq_sb[:], in_=wq_d[:])
                nc.sync.dma_start(out=bq_sb[:], in_=bq_d[:])
                emit_qk_dma(0, "q")
                emit_qk_proj(0, "k", fs=[0])
                emit_qk_proj(0, "q", rcs=[0], fs=[0])
                emit_scores(0, 0, 0)
                nc.sync.dma_start(out=wv_sb[:], in_=wv_d[:])
                emit_qk_proj(0, "k", fs=[1])
                emit_scores(0, 1, 0)
                emit_v_dma(0)
                emit_qk_proj(0, "q", rcs=[0], fs=[1])
                emit_qk_proj(0, "q", rcs=[1, 2, 3])
                nc.sync.dma_start(out=wo_sb[:], in_=wo_d[:])
                nc.sync.dma_start(out=id_sb[:], in_=id_d[:])
                emit_scores(0, 2, 0)
                emit_v_proj(0, 0)
                emit_v_proj(0, 1)
                # steady state: attention/outproj of b with proj work for b+1
                # sliced into ~1.7us pieces drained at several ladder points
                # per chunk, so no long PE block ever delays the next scores
                # (which would starve ACT, the critical engine)
                for b in range(BPC):
                    slices = []
                    if b + 1 < BPC:
                        alloc_batch(b + 1)
                        nb = b + 1
                        slices.append(lambda nb=nb: emit_qk_dma(nb, "q"))
                        for rc in range(S // 512):
                            for f in range(FCH):
                                slices.append(
                                    lambda nb=nb, rc=rc, f=f: emit_qk_proj(
                                        nb, "q", rcs=[rc], fs=[f]
                                    )
                                )
                        slices.append(lambda nb=nb: emit_qk_dma(nb, "k"))
                        for rc in range(S // 512):
                            for f in range(FCH):
                                slices.append(
                                    lambda nb=nb, rc=rc, f=f: emit_qk_proj(
                                        nb, "k", rcs=[rc], fs=[f]
                                    )
                                )
                        slices.append(lambda nb=nb: emit_v_dma(nb))
                        for k2 in range(NKV // 2):
                            slices.append(
                                lambda nb=nb, k2=k2: emit_v_proj(
                                    nb, kvts=[k2 * 2, k2 * 2 + 1]
                                )
                            )
                    sl = iter(slices)
                    left = len(slices)

                    def drain(n):
                        nonlocal left
                        for _ in range(n):
                            s = next(sl, None)
                            if s is None:
                                return
                            s()
                            left -= 1

                    for qc in range(NQC):
                        pre = b == 0 and qc == 0
                        # even share of remaining slices over remaining qcs
                        share = (left + (NQC - qc) - 1) // (NQC - qc)
                        if not pre and qc == 0:
                            emit_scores(b, 0, qc)
                        if not pre:
                            emit_scores(b, 1, qc)
                        emit_av(b, 0, qc)
                        if not pre:
                            emit_scores(b, 2, qc)
                        drain(share // 3)
                        emit_av(b, 1, qc)
                        emit_scores(b, 3, qc)
                        drain(share // 3)
                        emit_av(b, 2, qc)
                        emit_av(b, 3, qc)
                        drain(share - 2 * (share // 3))
                        if qc + 1 < NQC:
                            # peel next chunk's first scores ahead of outproj
                            # so ACT keeps a 2-group buffer at the boundary
                            emit_scores(b, 0, qc + 1)
                        emit_outproj(b, qc)

            for _ in range(iters):
                emit_iteration()

    nc.finalize()
    return nc


_NC_CACHE = {}


def get_nc(B=4, S=2048, lowering=False):
    key = (B, S, lowering)
    if key not in _NC_CACHE:
        _NC_CACHE[key] = build_nc(B, S, lowering)
    return _NC_CACHE[key]


def make_in_maps(value, key, query, Wv, bv, Wk, bk, Wq, bq, Wo, bo, B, S):
    ROWS = B * S
    bf = ml_dtypes.bfloat16
    f8 = ml_dtypes.float8_e4m3
    # fp8 chunk-major [128, NEC, ROWS] for q/k; bf16 feature-major for v
    qT8 = query.reshape(ROWS, EMBED).astype(f8).T.reshape(NEC, 128, ROWS)
    qT8 = np.ascontiguousarray(qT8.transpose(1, 0, 2))  # [128, NEC, ROWS]
    kT8 = key.reshape(ROWS, EMBED).astype(f8).T.reshape(NEC, 128, ROWS)
    kT8 = np.ascontiguousarray(kT8.transpose(1, 0, 2))
    vTh = np.ascontiguousarray(value.reshape(ROWS, EMBED).astype(bf).T)
    ident = np.eye(128, dtype=bf)
    in_maps = []
    for c in range(N_CORES):
        bg, hg = c // HPC, c % HPC
        rs = slice(bg * BPC * S, (bg + 1) * BPC * S)
        cs = slice(hg * DC, (hg + 1) * DC)

        def wchunks(W, dt):
            return np.ascontiguousarray(
                W[:, cs].astype(dt).reshape(NEC, 128, DC).transpose(1, 0, 2).reshape(128, NEC * DC)
            )

        in_maps.append(
            {
                "qT8": np.ascontiguousarray(qT8[:, :, rs]).reshape(128, -1),
                "kT8": np.ascontiguousarray(kT8[:, :, rs]).reshape(128, -1),
                "vT": np.ascontiguousarray(vTh[:, rs]),
                "wq8": wchunks(Wq, f8),
                "wk8": wchunks(Wk, f8),
                "wv": wchunks(Wv, bf),
                "bq": np.ascontiguousarray(
                    bq[cs].reshape(FCH, 128).T.astype(np.float32)
                ),
                "bk": np.ascontiguousarray(
                    bk[cs].reshape(FCH, 128).T.astype(np.float32)
                ),
                "wo": np.ascontiguousarray(
                    Wo[cs, :].astype(bf).reshape(FCH, 128, EMBED).transpose(1, 0, 2).reshape(128, FCH * EMBED)
                ),
                "ident": ident,
            }
        )
    return in_maps


def finish(results, Wv, bv, Wo, bo, B, S):
    const_row = (
        bv.astype(np.float32) @ Wo.astype(np.float32) + bo.astype(np.float32)
    )[None, :]
    out = np.empty((B * S, EMBED), np.float32)
    for bg in range(B // BPC):
        acc = results[bg * HPC]["out"].astype(np.float32)
        for hg in range(1, HPC):
            acc = acc + results[bg * HPC + hg]["out"].astype(np.float32)
        out[bg * BPC * S : (bg + 1) * BPC * S] = acc
    out += const_row
    return out.reshape(B, S, EMBED)


def kernel(value, key, query, Wv, bv, Wk, bk, Wq, bq, Wo, bo):
    B, S, _ = query.shape
    nc = get_nc(B, S)
    in_maps = make_in_maps(value, key, query, Wv, bv, Wk, bk, Wq, bq, Wo, bo, B, S)
    res = run_bass_kernel_spmd(nc, in_maps, list(range(N_CORES)))
    return finish(res.results, Wv, bv, Wo, bo, B, S)



# revision 35
# speedup vs baseline: 1.8070x; 1.1692x over previous
"""MultiHeadAttention TRN2 kernel — hybrid sharding: 2 batch-groups x 4
head-groups over 8 cores. Core c = (bg, hg) with bg = c//4, hg = c%4 owns
batches {2bg, 2bg+1} and heads {4hg..4hg+3} == feature columns
hg*256:(hg+1)*256 of Wq/Wk/Wv and rows hg*256:(hg+1)*256 of Wo.

vs pure head-parallel this halves the dominant DMA traffic (each core loads
x for 2 of 4 batches instead of all) while keeping per-core PE/ACT/DVE work
identical, and keeps >=2 local batches so projections of batch b+1 pipeline
under attention of batch b.

Device math (per core), all matmuls bf16 with f32 PSUM accumulation:
  qT/kT = (Wq_c^T x^T + bq_c)          feature-major [2x128, S] per batch
  v     = x^T^T Wv_c                   position-major, 65-wide blocks per
                                        (kv-tile, head); col 64 = ones so AV
                                        accumulates the softmax denominator
  scoresT[kv, q] = kT^T qT             per (b, h), exp via ACT with scale=1/8
  AV (position-major, full 128 PSUM partitions):
    ap[q, j] = sum_kv e[kv, q] v_aug[kv, j]
  attn_pm[q, d] = ap[q, d] / ap[q, 64]  per-partition scalar mul on DVE
  transpose attn_pm -> feature-major via PE identity transpose (f32)
  out_partial[q, e] = sum_hp attn_fm^T Wo_chunk   bf16 out, host sums the 4
                                        head-group partials per batch-group
"""

import sys

sys.path.insert(0, "/opt/trn_rl_repo")

import numpy as np
import ml_dtypes

import concourse.bass as bass
from concourse import bacc
import concourse.mybir as mybir
from concourse.tile import TileContext
from concourse.bass_utils import run_bass_kernel_spmd

BF16 = mybir.dt.bfloat16
F32 = mybir.dt.float32
F8 = mybir.dt.float8e4
AF = mybir.ActivationFunctionType
DR = mybir.MatmulPerfMode.DoubleRow

EMBED = 1024
HEADS = 16
HEAD_DIM = 64
N_CORES = 8
BPC = 2  # batches per core (batch-group size)
HPC = 4  # heads per core
DC = HPC * HEAD_DIM  # 256 feature columns per core
FCH = DC // 128  # feature chunks of 128
NEC = 8  # contraction chunks of 128 over EMBED


def build_nc(B=4, S=2048, lowering=False, iters=1, ablate=None, sps_mode="2x1024"):
    MB = 3 if sps_mode == "2x1024" else 2  # misc ring banks
    ROWS = BPC * S  # rows owned by this core's batch-group
    NQC = S // 512  # q chunks per (b, h)
    NKV = S // 128  # kv tiles per batch
    NKV2 = NKV // 2
    nc = bacc.Bacc("TRN2", target_bir_lowering=lowering)

    # q/k path in fp8e4 (chunk-major [128, NEC, ROWS] flattened) for
    # DoubleRow matmuls; v path stays bf16 feature-major.
    qT_d = nc.declare_dram_parameter("qT8", [128, NEC * ROWS], F8, isOutput=False)
    kT_d = nc.declare_dram_parameter("kT8", [128, NEC * ROWS], F8, isOutput=False)
    vT_d = nc.declare_dram_parameter("vT", [EMBED, ROWS], BF16, isOutput=False)
    wq_d = nc.declare_dram_parameter("wq8", [128, NEC * DC], F8, isOutput=False)
    wk_d = nc.declare_dram_parameter("wk8", [128, NEC * DC], F8, isOutput=False)
    wv_d = nc.declare_dram_parameter("wv", [128, NEC * DC], BF16, isOutput=False)
    bq_d = nc.declare_dram_parameter("bq", [128, FCH], F32, isOutput=False)
    bk_d = nc.declare_dram_parameter("bk", [128, FCH], F32, isOutput=False)
    wo_d = nc.declare_dram_parameter("wo", [128, FCH * EMBED], BF16, isOutput=False)
    id_d = nc.declare_dram_parameter("ident", [128, 128], BF16, isOutput=False)
    out_d = nc.declare_dram_parameter("out", [ROWS, EMBED], BF16, isOutput=True)

    with TileContext(nc) as tc:
        with (
            tc.tile_pool(name="const", bufs=1) as cpool,
            tc.tile_pool(name="big", bufs=2) as big,
            tc.tile_pool(name="xin", bufs=32) as xin,
            tc.tile_pool(name="expp", bufs=14) as expp,
            tc.tile_pool(name="apm", bufs=16) as apmp,
            tc.tile_pool(name="ev", bufs=8) as evp,
            tc.tile_pool(name="ot", bufs=4) as otp,
            tc.tile_pool(name="ps", bufs=1, space="PSUM") as ps,
        ):
            # --- weights / constants ---
            wq_sb = cpool.tile([128, NEC * DC], F8, tag="wq")
            wk_sb = cpool.tile([128, NEC * DC], F8, tag="wk")
            wv_sb = cpool.tile([128, NEC * DC], BF16, tag="wv")
            wo_sb = cpool.tile([128, FCH * EMBED], BF16, tag="wo")
            bq_sb = cpool.tile([128, FCH], F32, tag="bq")
            bk_sb = cpool.tile([128, FCH], F32, tag="bk")
            id_sb = cpool.tile([128, 128], BF16, tag="ident")

            # --- PSUM bank plan (8 banks; deps are BANK-granular!) ---
            # "sps" ring: 2 x [128,1536] f32 (3 banks each) so one exp
            # instruction covers 1536 columns (ACT per-instruction overhead
            # is ~270ns; fewer+bigger exps is the ACT win), double-buffered
            # so PE fills one slot while ACT drains the other.
            # "mr" ring: 2 x 1 bank rotated by ALL small psum users (AV
            # accumulators, proj/outproj outputs) in program order; each
            # user's WAR lands on the drain of the use two allocations back.
            qT_sb, kT_sb, v_sb = {}, {}, {}

            def alloc_batch(b):
                qT_sb[b] = big.tile([128, FCH * S], BF16, tag="qTs", name=f"qTs{b}")
                kT_sb[b] = big.tile([128, FCH * S], BF16, tag="kTs", name=f"kTs{b}")
                v_sb[b] = big.tile(
                    [128, NKV * HPC * 65], BF16, tag="vs", name=f"vs{b}"
                )
                ones_ap = v_sb[b][:].rearrange("p (n c) -> p n c", c=65)[:, :, 64:65]
                nc.vector.memset(ones_ap, 1.0)  # ones col (idx 64) per 65-block

            qk_xt = {}

            def emit_qk_dma(b, which):
                # fp8 chunk-pair tiles [128, 2, S//2] for DoubleRow
                src_d = qT_d if which == "q" else kT_d
                src3 = src_d[:, :].rearrange("p (e r) -> p e r", e=NEC)
                tiles = {}
                for hf in range(2):
                    for ep in range(NEC // 2):
                        t = xin.tile(
                            [128, S], F8, tag="xin", name=f"x{which}{b}e{ep}h{hf}"
                        )
                        nc.sync.dma_start(
                            out=t[:].rearrange("p (j c) -> p j c", j=2),
                            in_=src3[
                                :,
                                2 * ep : 2 * ep + 2,
                                b * S + hf * (S // 2) : b * S + (hf + 1) * (S // 2),
                            ],
                        )
                        tiles[(ep, hf)] = t
                qk_xt[(b, which)] = tiles

            qk_done = {}

            def emit_qk_proj(b, which, rcs=None, fs=None):
                wsb, bsb = (wq_sb, bq_sb) if which == "q" else (wk_sb, bk_sb)
                dst = (qT_sb if which == "q" else kT_sb)[b]
                if (b, which) not in qk_xt:
                    emit_qk_dma(b, which)
                tiles = qk_xt[(b, which)]
                w3 = wsb[:].rearrange("p (e c) -> p e c", e=NEC)
                rcph = max(1, S // 2 // 512)
                if rcs is None:
                    rcs = range(S // 512)
                if fs is None:
                    fs = range(FCH)
                done = qk_done.setdefault((b, which), set())
                for rc in rcs:
                    hf, off = rc // rcph, (rc % rcph) * 512
                    for f in fs:
                        ptt = ps.tile([128, 512], F32, tag="misc", bufs=MB, name="pt")
                        pt = ptt[:]
                        for ep in range(NEC // 2):
                            nc.tensor.matmul(
                                pt,
                                w3[:, 2 * ep : 2 * ep + 2, f * 128 : (f + 1) * 128],
                                tiles[(ep, hf)][:]
                                .rearrange("p (j c) -> p j c", j=2)[
                                    :, :, off : off + 512
                                ],
                                start=(ep == 0),
                                stop=(ep == NEC // 2 - 1),
                                perf_mode=DR,
                            )
                        nc.vector.tensor_scalar_add(
                            dst[:, f * S + rc * 512 : f * S + (rc + 1) * 512],
                            pt,
                            bsb[:, f : f + 1],
                        )
                        done.add((rc, f))
                if len(done) == (S // 512) * FCH:
                    del qk_xt[(b, which)]
                    del qk_done[(b, which)]

            def emit_v_dma(b):
                xt = {}
                for ec in range(NEC):
                    for hf in range(2):
                        t = xin.tile(
                            [128, S // 2], BF16, tag="xin", name=f"xv{b}e{ec}h{hf}"
                        )
                        nc.sync.dma_start(
                            out=t[:],
                            in_=vT_d[
                                ec * 128 : (ec + 1) * 128,
                                b * S + hf * (S // 2) : b * S + (hf + 1) * (S // 2),
                            ],
                        )
                        xt[(ec, hf)] = t
                v_xt[b] = xt

            v_done = {}

            def emit_v_proj(b, part=None, kvts=None):
                if b not in v_xt:
                    emit_v_dma(b)
                xt = v_xt[b]
                if kvts is None:
                    kvts = range(part * NKV // 2, (part + 1) * NKV // 2)
                done = v_done.setdefault(b, set())
                for kvt in kvts:
                    hf = kvt // (NKV // 2)
                    off = (kvt % (NKV // 2)) * 128
                    pvt = ps.tile([128, 256], F32, tag="misc", bufs=MB, name="pv")
                    pv = pvt[:]
                    for ec in range(NEC):
                        nc.tensor.matmul(
                            pv,
                            xt[(ec, hf)][:, off : off + 128],
                            wv_sb[:, ec * DC : (ec + 1) * DC],
                            start=(ec == 0),
                            stop=(ec == NEC - 1),
                        )
                    for h in range(HPC):
                        c0 = (kvt * HPC + h) * 65
                        nc.vector.tensor_copy(
                            v_sb[b][:, c0 : c0 + 64], pv[:, h * 64 : (h + 1) * 64]
                        )
                    done.add(kvt)
                if len(done) == NKV:
                    del v_xt[b]
                    del v_done[b]

            def emit_proj_part(b, part):
                if part == 0:
                    emit_qk_proj(b, "q")
                elif part == 1:
                    emit_qk_proj(b, "k")
                else:
                    emit_v_proj(b, part - 2)

            apm = {}
            et_store = {}
            # score/exp groups per (b,h,qc) on the double-buffered "sps"
            # ring; group layout set by sps_mode
            if sps_mode == "2x1024":
                SG = [(0, 2), (2, 2), (4, 2), (6, 2), (8, 2), (10, 2), (12, 2), (14, 2)]
            else:  # "32": alternating 1536/1024 slots in one 5-bank ring
                SG = [(0, 3), (3, 2), (5, 3), (8, 2), (10, 3), (13, 2), (15, 1)]
            GIDX, GOFF = [], []
            for gi, (k0, klen) in enumerate(SG):
                GIDX += [gi] * klen
                GOFF += list(range(klen))

            def emit_scores(b, h, qc):
                f, dh = h // 2, (h % 2) * 64
                qcol = qc * 512
                qTb, kTb = qT_sb[b], kT_sb[b]
                et = []
                for gi, (k0, klen) in enumerate(SG):
                    w = klen * 512
                    if sps_mode == "2x1024":
                        sps = ps.tile([128, 1024], F32, tag="sps", bufs=2, name="sps")
                    elif gi % 2 == 0:
                        sps = ps.tile([128, 1536], F32, tag="spsA", bufs=1, name="spsA")
                    else:
                        sps = ps.tile([128, 1024], F32, tag="spsB", bufs=1, name="spsB")
                    for j in range(klen):
                        if ablate in ("pe0", "both") and j % 2 == 1:
                            continue  # timing ablation: halve scores matmuls
                        kvt = k0 + j
                        nc.tensor.matmul(
                            sps[:, j * 512 : (j + 1) * 512],
                            kTb[dh : dh + 64, f * S + kvt * 128 : f * S + (kvt + 1) * 128],
                            qTb[dh : dh + 64, f * S + qcol : f * S + qcol + 512],
                            start=True,
                            stop=True,
                        )
                    e_t = expp.tile([128, w], BF16, tag="expp", name="et")
                    if ablate in ("act0", "both"):
                        # timing ablation: halve ACT busy (upper half stale)
                        nc.scalar.activation(
                            e_t[:, 0 : w // 2], sps[:, 0 : w // 2], AF.Exp, scale=0.125
                        )
                    else:
                        nc.scalar.activation(e_t[:], sps[:, 0:w], AF.Exp, scale=0.125)
                    et.append(e_t)
                et_store[h] = et

            def emit_av(b, h, qc):
                # AV position-major: per q-tile of 128, accumulate over 16 kv
                # tiles. Accumulators ap0/ap1 alternate in ob_t[0:130] so a
                # group never waits on the DVE drain of the previous group.
                hp, dh = h // 2, (h % 2) * 64
                vb = v_sb[b]
                et = et_store[h]
                for qt in range(4):
                    qoff = qt * 128
                    if h % 2 == 0:
                        apm[(hp, qt)] = apmp.tile(
                            [128, 128], BF16, tag="apm", name=f"apm{hp}_{qt}"
                        )
                    ap = ps.tile(
                        [128, 65], F32,
                        tag=("aps" if qt % 2 == 0 else "misc"),
                        bufs=(1 if qt % 2 == 0 else MB), name="aps",
                    )
                    nkv_eff = 4 if ablate == "av4" else NKV
                    for kvt in range(nkv_eff):
                        g, j = GIDX[kvt], GOFF[kvt]
                        c0 = (kvt * HPC + h) * 65
                        nc.tensor.matmul(
                            ap[:],
                            et[g][:, j * 512 + qoff : j * 512 + qoff + 128],
                            vb[:, c0 : c0 + 65],
                            start=(kvt == 0),
                            stop=(kvt == nkv_eff - 1),
                        )
                    rec = evp.tile([128, 1], F32, tag="rec", name="rec")
                    nc.vector.reciprocal(rec[:], ap[:, 64:65])
                    nc.vector.tensor_scalar_mul(
                        apm[(hp, qt)][:, dh : dh + 64], ap[:, 0:64], rec[:, 0:1]
                    )

            def emit_outproj(b, qc):
                for qt in range(4):
                    qcol = qc * 512 + qt * 128
                    afm = {}
                    for hp in range(FCH):
                        tp = ps.tile([128, 128], BF16, tag="misc", bufs=MB, name="tp")
                        nc.tensor.transpose(tp[:], apm[(hp, qt)][:], id_sb[:])
                        afm[hp] = evp.tile([128, 128], BF16, tag="afm", name="afm")
                        nc.vector.tensor_copy(afm[hp][:], tp[:])
                    ot = otp.tile([128, EMBED], BF16, tag="ot", name="ot")
                    for en in range(2):
                        po = ps.tile([128, 512], F32, tag="misc", bufs=MB, name="po")
                        for hp in range(FCH):
                            nc.tensor.matmul(
                                po[:],
                                afm[hp][:],
                                wo_sb[:, hp * EMBED + en * 512 : hp * EMBED + en * 512 + 512],
                                start=(hp == 0),
                                stop=(hp == FCH - 1),
                            )
                        nc.vector.tensor_copy(ot[:, en * 512 : (en + 1) * 512], po[:])
                    nc.sync.dma_start(
                        out=out_d[b * S + qcol : b * S + qcol + 128, :], in_=ot[:]
                    )

            v_xt = {}

            def emit_iteration():
                # prologue: heads 0/1 only need feature-chunk 0 of kT/qT, so
                # emit k-proj f0 -> q-proj rc0 f0 -> first scores as early as
                # possible (first exp ~16us in), filling the rest of the
                # projections and the remaining weight DMAs under the first
                # exps. k-proj weights first: they gate the first scores.
                nc.sync.dma_start(out=wk_sb[:], in_=wk_d[:])
                nc.sync.dma_start(out=bk_sb[:], in_=bk_d[:])
                alloc_batch(0)
                emit_qk_dma(0, "k")
                nc.sync.dma_start(out=wq_sb[:], in_=wq_d[:])
                nc.sync.dma_start(out=bq_sb[:], in_=bq_d[:])
                emit_qk_dma(0, "q")
                emit_qk_proj(0, "k", fs=[0])
                emit_qk_proj(0, "q", rcs=[0], fs=[0])
                emit_scores(0, 0, 0)
                nc.sync.dma_start(out=wv_sb[:], in_=wv_d[:])
                emit_qk_proj(0, "k", fs=[1])
                emit_scores(0, 1, 0)
                emit_v_dma(0)
                emit_qk_proj(0, "q", rcs=[0], fs=[1])
                emit_qk_proj(0, "q", rcs=[1, 2, 3])
                nc.sync.dma_start(out=wo_sb[:], in_=wo_d[:])
                nc.sync.dma_start(out=id_sb[:], in_=id_d[:])
                emit_scores(0, 2, 0)
                emit_v_proj(0, 0)
                emit_v_proj(0, 1)
                # steady state: attention/outproj of b with proj work for b+1
                # sliced into ~1.7us pieces drained at several ladder points
                # per chunk, so no long PE block ever delays the next scores
                # (which would starve ACT, the critical engine)
                for b in range(BPC):
                    slices = []
                    if b + 1 < BPC:
                        alloc_batch(b + 1)
                        nb = b + 1
                        slices.append(lambda nb=nb: emit_qk_dma(nb, "q"))
                        for rc in range(S // 512):
                            for f in range(FCH):
                                slices.append(
                                    lambda nb=nb, rc=rc, f=f: emit_qk_proj(
                                        nb, "q", rcs=[rc], fs=[f]
                                    )
                                )
                        slices.append(lambda nb=nb: emit_qk_dma(nb, "k"))
                        for rc in range(S // 512):
                            for f in range(FCH):
                                slices.append(
                                    lambda nb=nb, rc=rc, f=f: emit_qk_proj(
                                        nb, "k", rcs=[rc], fs=[f]
                                    )
                                )
                        slices.append(lambda nb=nb: emit_v_dma(nb))
                        for k2 in range(NKV // 2):
                            slices.append(
                                lambda nb=nb, k2=k2: emit_v_proj(
                                    nb, kvts=[k2 * 2, k2 * 2 + 1]
                                )
                            )
                    sl = iter(slices)
                    left = len(slices)

                    def drain(n):
                        nonlocal left
                        for _ in range(n):
                            s = next(sl, None)
                            if s is None:
                                return
                            s()
                            left -= 1

                    for qc in range(NQC):
                        pre = b == 0 and qc == 0
                        # even share of remaining slices over remaining qcs
                        share = (left + (NQC - qc) - 1) // (NQC - qc)
                        if not pre and qc == 0:
                            emit_scores(b, 0, qc)
                        if not pre:
                            emit_scores(b, 1, qc)
                        emit_av(b, 0, qc)
                        if not pre:
                            emit_scores(b, 2, qc)
                        drain(share // 3)
                        emit_av(b, 1, qc)
                        emit_scores(b, 3, qc)
                        drain(share // 3)
                        emit_av(b, 2, qc)
                        emit_av(b, 3, qc)
                        drain(share - 2 * (share // 3))
                        if qc + 1 < NQC:
                            # peel next chunk's first scores ahead of outproj
                            # so ACT keeps a 2-group buffer at the boundary
                            emit_scores(b, 0, qc + 1)
                        emit_outproj(b, qc)

            for _ in range(iters):
                emit_iteration()

    nc.finalize()
    return nc


_NC_CACHE = {}


def get_nc(B=4, S=2048, lowering=False):
    key = (B, S, lowering)
    if key not in _NC_CACHE:
        _NC_CACHE[key] = build_nc(B, S, lowering)
    return _NC_CACHE[key]


def make_in_maps(value, key, query, Wv, bv, Wk, bk, Wq, bq, Wo, bo, B, S):
    ROWS = B * S
    bf = ml_dtypes.bfloat16
    f8 = ml_dtypes.float8_e4m3
    # fp8 chunk-major [128, NEC, ROWS] for q/k; bf16 feature-major for v
    qT8 = query.reshape(ROWS, EMBED).astype(f8).T.reshape(NEC, 128, ROWS)
    qT8 = np.ascontiguousarray(qT8.transpose(1, 0, 2))  # [128, NEC, ROWS]
    kT8 = key.reshape(ROWS, EMBED).astype(f8).T.reshape(NEC, 128, ROWS)
    kT8 = np.ascontiguousarray(kT8.transpose(1, 0, 2))
    vTh = np.ascontiguousarray(value.reshape(ROWS, EMBED).astype(bf).T)
    ident = np.eye(128, dtype=bf)
    in_maps = []
    for c in range(N_CORES):
        bg, hg = c // HPC, c % HPC
        rs = slice(bg * BPC * S, (bg + 1) * BPC * S)
        cs = slice(hg * DC, (hg + 1) * DC)

        def wchunks(W, dt):
            return np.ascontiguousarray(
                W[:, cs].astype(dt).reshape(NEC, 128, DC).transpose(1, 0, 2).reshape(128, NEC * DC)
            )

        in_maps.append(
            {
                "qT8": np.ascontiguousarray(qT8[:, :, rs]).reshape(128, -1),
                "kT8": np.ascontiguousarray(kT8[:, :, rs]).reshape(128, -1),
                "vT": np.ascontiguousarray(vTh[:, rs]),
                "wq8": wchunks(Wq, f8),
                "wk8": wchunks(Wk, f8),
                "wv": wchunks(Wv, bf),
                "bq": np.ascontiguousarray(
                    bq[cs].reshape(FCH, 128).T.astype(np.float32)
                ),
                "bk": np.ascontiguousarray(
                    bk[cs].reshape(FCH, 128).T.astype(np.float32)
                ),
                "wo": np.ascontiguousarray(
                    Wo[cs, :].astype(bf).reshape(FCH, 128, EMBED).transpose(1, 0, 2).reshape(128, FCH * EMBED)
                ),
                "ident": ident,
            }
        )
    return in_maps


def finish(results, Wv, bv, Wo, bo, B, S):
    const_row = (
        bv.astype(np.float32) @ Wo.astype(np.float32) + bo.astype(np.float32)
    )[None, :]
    out = np.empty((B * S, EMBED), np.float32)
    for bg in range(B // BPC):
        acc = results[bg * HPC]["out"].astype(np.float32)
        for hg in range(1, HPC):
            acc = acc + results[bg * HPC + hg]["out"].astype(np.float32)
        out[bg * BPC * S : (bg + 1) * BPC * S] = acc
    out += const_row
    return out.reshape(B, S, EMBED)


def kernel(value, key, query, Wv, bv, Wk, bk, Wq, bq, Wo, bo):
    B, S, _ = query.shape
    nc = get_nc(B, S)
    in_maps = make_in_maps(value, key, query, Wv, bv, Wk, bk, Wq, bq, Wo, bo, B, S)
    res = run_bass_kernel_spmd(nc, in_maps, list(range(N_CORES)))
    return finish(res.results, Wv, bv, Wo, bo, B, S)

